# revision 3
# baseline (speedup 1.0000x reference)
"""CrossAttention (B=2, N=M=2048, 16 heads x 64) on 8 TRN2 NeuronCores.

Sharding: data-parallel over batch (2) x tensor-parallel over heads (4 per
core). Each core computes q/k/v projections for its 4 heads, streaming
softmax(QK^T)V in a transposed (feature-major) layout, and a partial output
projection against its row-slice of Wo. Partial outputs are summed on host.

v2: fp8 (e4m3) DoubleRow matmuls where precision allows.
- Projections run as 3-term hi/lo fp8 products (x_hi*W_hi + x_hi*W_lo +
  x_lo*W_hi, lo*lo dropped): 12 DoubleRow instrs replace 8 bf16 instrs
  (0.75x PE time) at ~bf16 accuracy. The hi/lo splits of x/context/W are
  computed on host (free) with power-of-2 pre-scales so both magnitudes sit
  in e4m3's normal range.
- QK^T is one-sided fp8: q is stored exactly as fp8 hi+lo (device split),
  k is plain fp8. One DoubleRow instr per (head, ctx-tile, 512 query cols)
  contracts hi and lo against a stride-0-broadcast k tile: 2x fewer PE
  cycles than bf16, measured end-to-end rel err ~0.8% (budget 2%).
- PV and the output projection stay bf16 (fp8 there costs ~2.5% error).

Softmax: logits are small, exp() without max-subtraction is safe. The
denominator comes free from a ones-column appended to V (PSUM row 64 of the
PV matmul accumulates sum(exp)).

Schedule: activation (exp over 16.8M logits/core) is the binding engine
(~133us); everything else is paced to hide under it. PV/projection/output
work units ride a single FIFO filler queue drained between QK+exp pairs,
weighted by estimated PE-ns. Input DMA is ordered so the first q/k chains
start ~5us in.
"""

import sys

if "/opt/trn_rl_repo" not in sys.path:
    sys.path.insert(0, "/opt/trn_rl_repo")

from collections import deque

import ml_dtypes
import numpy as np

import concourse.bass as bass
import concourse.mybir as mybir
import concourse.tile as tile
from concourse import bacc
from concourse.bass_utils import run_bass_kernel_spmd

HEADS = 16
DH = 64
QD = 1024  # query/context feature dim
NN = 2048  # query tokens
MM = 2048  # context tokens
NCORES = 8
HPC = HEADS // (NCORES // 2)  # 4 heads per core
HD = HPC * DH  # 256 inner cols per core
KT = QD // 128  # 8 contraction tiles for projections
TT = MM // 128  # 16 context-token tiles

BF = mybir.dt.bfloat16
F8 = mybir.dt.float8e4
F32 = mybir.dt.float32
DR = mybir.MatmulPerfMode.DoubleRow

SX = 8.0  # host pre-scale on x/context before fp8 hi/lo split
SW = 256.0  # host pre-scale on Wq/Wk/Wv
SQ = 2.0 ** -9  # device descale: q8/k8 carry 4x their true value
ESC = 0.125 / 16.0  # exp scale: dh^-0.5 corrected for the 16x in q8*k8
SV = 1.0 / (SX * SW)  # device descale for v

# rough per-unit PE-ns estimates for filler pacing
EST_QKCH = 12 * 107  # q/k projection chain (12 DoubleRow at N=512)
EST_VCH = 12 * 54  # v projection chain (12 DoubleRow at N=256)
EST_PV = 2 * 214  # one pv mm unit (2 bf16 at N=512)
EST_FIN = 2 * 214  # one output-projection unit (2 bf16 at N=512)
FILL_NS = 2 * (1024 * 0.834 + 185) - 4 * 107  # ACT time minus QK time per tt

_CACHE = {}


def _build():
    nc = bacc.Bacc("TRN2", target_bir_lowering=False, debug=False)
    x8 = nc.declare_dram_parameter("x8", [QD, 2, NN], F8, isOutput=False)
    c8 = nc.declare_dram_parameter("c8", [QD, 2, MM], F8, isOutput=False)
    wq8 = nc.declare_dram_parameter("wq8", [QD, 2, HD], F8, isOutput=False)
    wk8 = nc.declare_dram_parameter("wk8", [QD, 2, HD], F8, isOutput=False)
    wv8 = nc.declare_dram_parameter("wv8", [QD, 2, HD], F8, isOutput=False)
    wo = nc.declare_dram_parameter("wo", [HD, QD], BF, isOutput=False)
    out = nc.declare_dram_parameter("out", [QD, NN], F32, isOutput=True)

    with tile.TileContext(nc) as tc:
        _emit(tc, x8, c8, wq8, wk8, wv8, wo, out)
    nc.compile()
    return nc


def _emit(tc, x8, c8, wq8, wk8, wv8, wo, out):
    nc = tc.nc
    Exp = mybir.ActivationFunctionType.Exp
    mult = mybir.AluOpType.mult
    sub = mybir.AluOpType.subtract

    from contextlib import ExitStack
    ctx = ExitStack()
    persist = ctx.enter_context(tc.tile_pool(name="persist", bufs=1))
    xs = persist.tile([128, KT, 2, NN], F8, tag="xs")
    cs = persist.tile([128, KT, 2, MM], F8, tag="cs")
    wqs = persist.tile([128, KT, 2, HD], F8, tag="wqs")
    wks = persist.tile([128, KT, 2, HD], F8, tag="wks")
    wvs = persist.tile([128, KT, 2, HD], F8, tag="wvs")
    wos = persist.tile([128, 2, QD], BF, tag="wos")
    qs = persist.tile([128, 2, 2, NN], F8, tag="qs")  # [2heads*64d, hp, hi/lo, tok]
    ks = persist.tile([128, 2, MM], F8, tag="ks")  # [2heads*64d, hp, tok]
    vs = persist.tile([128, TT, HPC, DH + 1], BF, tag="vs")  # v + ones col
    pvs = persist.tile([128, 2, NN], BF, tag="pvs")  # normalized attnV^T

    qkp = ctx.enter_context(tc.tile_pool(name="qk_ps", bufs=2, space="PSUM"))
    pvp = ctx.enter_context(tc.tile_pool(name="pv_ps", bufs=2, space="PSUM"))
    projp = ctx.enter_context(tc.tile_pool(name="proj_ps", bufs=2, space="PSUM"))
    expp = ctx.enter_context(tc.tile_pool(name="expp", bufs=37))
    outp = ctx.enter_context(tc.tile_pool(name="outp", bufs=2))
    nrm = ctx.enter_context(tc.tile_pool(name="nrm", bufs=4))

    # ---- input DMA, ordered so the prologue chains can start ASAP ----
    def dma_w(dst, src):
        for k in range(KT):
            nc.sync.dma_start(dst[:, k, :, :], src[k * 128:(k + 1) * 128, :, :])

    def dma_xc(dst, src, blk):
        c0 = blk * 512
        for k in range(KT):
            nc.sync.dma_start(dst[:, k, :, c0:c0 + 512],
                              src[k * 128:(k + 1) * 128, :, c0:c0 + 512])

    dma_w(wqs, wq8)
    dma_xc(xs, x8, 0)
    dma_xc(xs, x8, 1)
    dma_w(wks, wk8)
    dma_xc(cs, c8, 0)
    dma_w(wvs, wv8)
    for blk in range(1, 4):
        dma_xc(cs, c8, blk)
    dma_xc(xs, x8, 2)
    dma_xc(xs, x8, 3)
    for t in range(2):
        nc.sync.dma_start(wos[:, t, :], wo[t * 128:(t + 1) * 128, :])
    nc.gpsimd.memset(vs[:, :, :, DH:DH + 1], 1.0)

    # ---- projection chains: 12-instr 3-term hi/lo fp8 DoubleRow ----
    # products: (W_hi,x_hi), (W_lo,x_hi), (W_hi,x_lo); each instr packs the
    # same term for two adjacent k-tiles.
    TERMS = ((0, 0), (1, 0), (0, 1))

    def proj_mms(ps, w, wcols, src, scols):
        n = 0
        for whl, xhl in TERMS:
            for i in range(KT // 2):
                nc.tensor.matmul(
                    ps,
                    lhsT=w[:, 2 * i:2 * i + 2, whl, wcols],
                    rhs=src[:, 2 * i:2 * i + 2, xhl, scols],
                    start=(n == 0), stop=(n == 3 * KT // 2 - 1),
                    perf_mode=DR,
                )
                n += 1

    def q_chain(jb, i4):
        # q for head-pair jb, tokens [i4*512, +512); store exact fp8 hi+lo
        ps = projp.tile([128, 512], F32, tag="proj", name="ps")
        cc = slice(i4 * 512, (i4 + 1) * 512)
        proj_mms(ps[:, :], wqs, slice(jb * 128, (jb + 1) * 128), xs, cc)
        nc.vector.tensor_scalar_mul(qs[:, jb, 0, cc], ps[:, :], SQ)
        nc.vector.scalar_tensor_tensor(qs[:, jb, 1, cc], ps[:, :], SQ,
                                       qs[:, jb, 0, cc], mult, sub)

    def k_chain(jb, i4):
        ps = projp.tile([128, 512], F32, tag="proj", name="ps")
        cc = slice(i4 * 512, (i4 + 1) * 512)
        proj_mms(ps[:, :], wks, slice(jb * 128, (jb + 1) * 128), cs, cc)
        nc.vector.tensor_scalar_mul(ks[:, jb, cc], ps[:, :], SQ)

    def v_chain(tt):
        # v for one context-token tile (token-major): [128 tok, HPC, DH]
        ps = projp.tile([128, HPC, DH], F32, tag="proj", name="ps")
        n = 0
        for whl, xhl in TERMS:
            for i in range(KT // 2):
                nc.tensor.matmul(
                    ps[:, :, :],
                    lhsT=cs[:, 2 * i:2 * i + 2, whl, tt * 128:(tt + 1) * 128],
                    rhs=wvs[:, 2 * i:2 * i + 2, xhl, :],
                    start=(n == 0), stop=(n == 3 * KT // 2 - 1),
                    perf_mode=DR,
                )
                n += 1
        nc.vector.tensor_scalar_mul(vs[:, tt, :, 0:DH], ps[:, :, :], SV)

    # ---- filler queue ----
    queue = deque()

    def final_unit(ib, ob):
        fp = projp.tile([128, 512], F32, tag="proj", name="fp")
        for t2 in range(2):
            nc.tensor.matmul(
                fp[:, :],
                lhsT=wos[:, t2, ob * 128:(ob + 1) * 128],
                rhs=pvs[:, t2, ib * 512:(ib + 1) * 512],
                start=(t2 == 0), stop=(t2 == 1),
            )
        ot = outp.tile([128, 512], F32, tag="ot", name="ot")
        nc.vector.tensor_copy(ot[:, :], fp[:, :])
        nc.sync.dma_start(out[ob * 128:(ob + 1) * 128, ib * 512:(ib + 1) * 512], ot[:, :])

    def push_finals(ib):
        for ob in range(QD // 128):
            queue.append((EST_FIN, lambda ib=ib, ob=ob: final_unit(ib, ob)))

    def make_pv(hp, ib2, after_norms=()):
        # pv mm/norm units for one attn call, pushed per-tile as exp lands
        cell = {}

        def mm(tt, h01, e):
            if tt == 0:
                cell[h01] = [pvp.tile([DH + 1, 512], F32, tag="pv", name="pv")
                             for _ in range(2)]
            for i01 in range(2):
                nc.tensor.matmul(
                    cell[h01][i01][:, :],
                    lhsT=vs[:, tt, 2 * hp + h01, :],
                    rhs=e[:, i01 * 512:(i01 + 1) * 512],
                    start=(tt == 0), stop=(tt == TT - 1),
                )

        def norm(h01):
            for i01 in range(2):
                p = cell[h01][i01]
                c0 = ib2 * 1024 + i01 * 512
                rc = nrm.tile([1, 512], F32, tag="rc", name="rc")
                nc.vector.reciprocal(rc[:, :], p[64:65, :])
                rep = nrm.tile([64, 512], F32, tag="rep", name="rep")
                nc.gpsimd.partition_broadcast(rep[:, :], rc[:, :])
                nc.vector.tensor_tensor(
                    pvs[h01 * 64:(h01 + 1) * 64, hp, c0:c0 + 512],
                    p[0:64, :], rep[:, :], mult)

        def push(tt, es):
            for h01 in range(2):
                queue.append((EST_PV, lambda tt=tt, h01=h01, e=es[(tt, h01)]: mm(tt, h01, e)))
            if tt == TT - 1:
                for h01 in range(2):
                    queue.append((0, lambda h01=h01: norm(h01)))
                for fn in after_norms:
                    fn()

        return push

    def attn(hp, ib2, push_pv):
        # per ctx tile: 4 one-sided fp8 DoubleRow QK matmuls + 2 exp passes;
        # drain the filler queue between tiles, paced by estimated PE-ns.
        budget = 0.0
        es = {}
        for tt in range(TT):
            qk0 = qkp.tile([128, 1024], F32, tag="qk", name="qk0")
            qk1 = qkp.tile([128, 1024], F32, tag="qk", name="qk1")
            for h01, qk in ((0, qk0), (1, qk1)):
                lhsT = (ks[h01 * 64:(h01 + 1) * 64, hp, tt * 128:(tt + 1) * 128]
                        .unsqueeze(1).broadcast_to([64, 2, 128]))
                for i01 in range(2):
                    c0 = ib2 * 1024 + i01 * 512
                    nc.tensor.matmul(
                        qk[:, i01 * 512:(i01 + 1) * 512],
                        lhsT=lhsT,
                        rhs=qs[h01 * 64:(h01 + 1) * 64, hp, :, c0:c0 + 512],
                        start=True, stop=True,
                        perf_mode=DR,
                    )
            e0 = expp.tile([128, 1024], BF, tag="exp", name="e0")
            nc.scalar.activation(e0[:, :], qk0[:, :], Exp, scale=ESC)
            e1 = expp.tile([128, 1024], BF, tag="exp", name="e1")
            nc.scalar.activation(e1[:, :], qk1[:, :], Exp, scale=ESC)
            es[(tt, 0)], es[(tt, 1)] = e0, e1
            push_pv(tt, es)
            budget += FILL_NS
            while queue and budget > 0:
                est, fn = queue.popleft()
                fn()
                budget -= max(est, 1)

    # ---- schedule ----
    # prologue: only what attn(0,0)'s first tiles need
    q_chain(0, 0)
    q_chain(0, 1)
    k_chain(0, 0)
    for i4 in range(1, 4):
        queue.append((EST_QKCH, lambda i4=i4: k_chain(0, i4)))
    for tt in range(TT):
        queue.append((EST_VCH, lambda tt=tt: v_chain(tt)))
    for i4 in range(2, 4):
        queue.append((EST_QKCH, lambda i4=i4: q_chain(0, i4)))

    attn(0, 0, make_pv(0, 0))
    for i4 in range(2):
        queue.append((EST_QKCH, lambda i4=i4: q_chain(1, i4)))
    for i4 in range(4):
        queue.append((EST_QKCH, lambda i4=i4: k_chain(1, i4)))
    attn(0, 1, make_pv(0, 1))
    for i4 in range(2, 4):
        queue.append((EST_QKCH, lambda i4=i4: q_chain(1, i4)))
    attn(1, 0, make_pv(1, 0, after_norms=(lambda: push_finals(0), lambda: push_finals(1))))
    attn(1, 1, make_pv(1, 1, after_norms=(lambda: push_finals(2), lambda: push_finals(3))))
    while queue:
        est, fn = queue.popleft()
        fn()
    ctx.close()


def _hilo(a):
    """[R, C] f32 -> [R, 2, C] fp8 e4m3 (hi, residual lo)."""
    f8 = ml_dtypes.float8_e4m3
    a = np.ascontiguousarray(a, np.float32)
    h = a.astype(f8)
    l = (a - h.astype(np.float32)).astype(f8)
    return np.ascontiguousarray(np.stack([h, l], axis=1))


def _inputs_for_core(c, x, context, Wq, Wk, Wv, Wo):
    b, g = c // (NCORES // 2), c % (NCORES // 2)
    sl = slice(g * HD, (g + 1) * HD)
    key = ("xc", b)
    if key not in _CACHE:
        _CACHE[key] = (_hilo(x[b].T * SX), _hilo(context[b].T * SX))
    x8b, c8b = _CACHE[key]
    return {
        "x8": x8b,
        "c8": c8b,
        "wq8": _hilo(Wq[:, sl] * SW),
        "wk8": _hilo(Wk[:, sl] * SW),
        "wv8": _hilo(Wv[:, sl] * SW),
        "wo": np.ascontiguousarray(Wo[sl, :]).astype(ml_dtypes.bfloat16),
    }


def kernel(x, context, Wq, Wk, Wv, Wo, bo):
    x = np.asarray(x, np.float32)
    context = np.asarray(context, np.float32)
    if "nc" not in _CACHE:
        _CACHE["nc"] = _build()
    _CACHE.pop(("xc", 0), None)
    _CACHE.pop(("xc", 1), None)
    nc = _CACHE["nc"]
    in_maps = [
        _inputs_for_core(c, x, context, np.asarray(Wq), np.asarray(Wk),
                         np.asarray(Wv), np.asarray(Wo))
        for c in range(NCORES)
    ]
    res = run_bass_kernel_spmd(nc, in_maps, list(range(NCORES))).results
    B = x.shape[0]
    G = NCORES // B
    outp = np.empty((B, NN, QD), np.float32)
    for b in range(B):
        acc = res[b * G]["out"].astype(np.float32)
        for g in range(1, G):
            acc = acc + res[b * G + g]["out"]
        outp[b] = acc.T + np.asarray(bo, np.float32)[None, :]
    return outp


# revision 9
# speedup vs baseline: 1.0464x; 1.0464x over previous
"""CrossAttention (B=2, N=M=2048, 16 heads x 64) on 8 TRN2 NeuronCores.

Sharding: data-parallel over batch (2) x tensor-parallel over heads (4 per
core). Each core computes q/k/v projections for its 4 heads, streaming
softmax(QK^T)V in a transposed (feature-major) layout, and a partial output
projection against its row-slice of Wo. Partial outputs are summed on host.

v2: fp8 (e4m3) DoubleRow matmuls where precision allows.
- Projections run as 3-term hi/lo fp8 products (x_hi*W_hi + x_hi*W_lo +
  x_lo*W_hi, lo*lo dropped): 12 DoubleRow instrs replace 8 bf16 instrs
  (0.75x PE time) at ~bf16 accuracy. The hi/lo splits of x/context/W are
  computed on host (free) with power-of-2 pre-scales so both magnitudes sit
  in e4m3's normal range.
- QK^T is one-sided fp8: q is stored exactly as fp8 hi+lo (device split),
  k is plain fp8. One DoubleRow instr per (head, ctx-tile, 512 query cols)
  contracts hi and lo against a stride-0-broadcast k tile: 2x fewer PE
  cycles than bf16, measured end-to-end rel err ~0.8% (budget 2%).
- PV and the output projection stay bf16 (fp8 there costs ~2.5% error).

Softmax: logits are small, exp() without max-subtraction is safe. The
denominator comes free from a ones-column appended to V (PSUM row 64 of the
PV matmul accumulates sum(exp)).

Schedule: activation (exp over 16.8M logits/core) is the binding engine
(~133us); everything else is paced to hide under it. PV/projection/output
work units ride a single FIFO filler queue drained between QK+exp pairs,
weighted by estimated PE-ns. Input DMA is ordered so the first q/k chains
start ~5us in.
"""

import sys

if "/opt/trn_rl_repo" not in sys.path:
    sys.path.insert(0, "/opt/trn_rl_repo")

from collections import deque

import ml_dtypes
import numpy as np

import concourse.bass as bass
import concourse.mybir as mybir
import concourse.tile as tile
from concourse import bacc
from concourse.bass_utils import run_bass_kernel_spmd

HEADS = 16
DH = 64
QD = 1024  # query/context feature dim
NN = 2048  # query tokens
MM = 2048  # context tokens
NCORES = 8
HPC = HEADS // (NCORES // 2)  # 4 heads per core
HD = HPC * DH  # 256 inner cols per core
KT = QD // 128  # 8 contraction tiles for projections
TT = MM // 128  # 16 context-token tiles

BF = mybir.dt.bfloat16
F8 = mybir.dt.float8e4
F32 = mybir.dt.float32
DR = mybir.MatmulPerfMode.DoubleRow

SX = 8.0  # host pre-scale on x/context before fp8 hi/lo split
SW = 256.0  # host pre-scale on Wq/Wk/Wv
SQ = 2.0 ** -9  # device descale: q8/k8 carry 4x their true value
ESC = 0.125 / 16.0  # exp scale: dh^-0.5 corrected for the 16x in q8*k8
SV = 1.0 / (SX * SW)  # device descale for v

# rough per-unit PE-ns estimates for filler pacing
EST_QKCH = 12 * 107  # q/k projection chain (12 DoubleRow at N=512)
EST_VCH = 12 * 54  # v projection chain (12 DoubleRow at N=256)
EST_PV = 2 * 214  # one pv mm unit (2 bf16 at N=512)
EST_FIN = 2 * 214  # one output-projection unit (2 bf16 at N=512)
EST_NORM = 2500  # norm unit: no PE work, but gates the pv PSUM buf reuse
FILL_NS = 2 * (1024 * 0.834 + 185) - 4 * 107  # ACT time minus QK time per tt

_CACHE = {}


def _build():
    nc = bacc.Bacc("TRN2", target_bir_lowering=False, debug=False)
    x8 = nc.declare_dram_parameter("x8", [QD, 2, NN], F8, isOutput=False)
    c8 = nc.declare_dram_parameter("c8", [QD, 2, MM], F8, isOutput=False)
    wq8 = nc.declare_dram_parameter("wq8", [QD, 2, HD], F8, isOutput=False)
    wk8 = nc.declare_dram_parameter("wk8", [QD, 2, HD], F8, isOutput=False)
    wv8 = nc.declare_dram_parameter("wv8", [QD, 2, HD], F8, isOutput=False)
    wo = nc.declare_dram_parameter("wo", [HD, QD], BF, isOutput=False)
    out = nc.declare_dram_parameter("out", [QD, NN], F32, isOutput=True)

    with tile.TileContext(nc) as tc:
        _emit(tc, x8, c8, wq8, wk8, wv8, wo, out)
    nc.compile()
    return nc


def _emit(tc, x8, c8, wq8, wk8, wv8, wo, out):
    nc = tc.nc
    Exp = mybir.ActivationFunctionType.Exp
    mult = mybir.AluOpType.mult
    sub = mybir.AluOpType.subtract

    from contextlib import ExitStack
    ctx = ExitStack()
    persist = ctx.enter_context(tc.tile_pool(name="persist", bufs=1))
    xs = persist.tile([128, KT, 2, NN], F8, tag="xs")
    cs = persist.tile([128, KT, 2, MM], F8, tag="cs")
    wqs = persist.tile([128, KT, 2, HD], F8, tag="wqs")
    wks = persist.tile([128, KT, 2, HD], F8, tag="wks")
    wvs = persist.tile([128, KT, 2, HD], F8, tag="wvs")
    wos = persist.tile([128, 2, QD], BF, tag="wos")
    qs = persist.tile([128, 2, 2, NN], F8, tag="qs")  # [2heads*64d, hp, hi/lo, tok]
    ks = persist.tile([128, 2, MM], F8, tag="ks")  # [2heads*64d, hp, tok]
    vs = persist.tile([128, TT, HPC, DH + 1], BF, tag="vs")  # v + ones col
    pvs = persist.tile([128, 2, NN], BF, tag="pvs")  # normalized attnV^T

    qkp = ctx.enter_context(tc.tile_pool(name="qk_ps", bufs=2, space="PSUM"))
    pvp = ctx.enter_context(tc.tile_pool(name="pv_ps", bufs=2, space="PSUM"))
    projp = ctx.enter_context(tc.tile_pool(name="proj_ps", bufs=2, space="PSUM"))
    expp = ctx.enter_context(tc.tile_pool(name="expp", bufs=39))
    outp = ctx.enter_context(tc.tile_pool(name="outp", bufs=2))
    nrm = ctx.enter_context(tc.tile_pool(name="nrm", bufs=4))

    # ---- input DMA, ordered so the prologue chains can start ASAP ----
    def dma_w(dst, src):
        for k in range(KT):
            nc.sync.dma_start(dst[:, k, :, :], src[k * 128:(k + 1) * 128, :, :])

    def dma_xc(dst, src, blk):
        c0 = blk * 512
        for k in range(KT):
            nc.sync.dma_start(dst[:, k, :, c0:c0 + 512],
                              src[k * 128:(k + 1) * 128, :, c0:c0 + 512])

    # order matches first-consumer time: prologue q/k chains, then v chains
    # and the remaining k/q chains as they pop from the filler queue
    dma_w(wqs, wq8)
    dma_w(wks, wk8)
    dma_xc(xs, x8, 0)
    dma_xc(xs, x8, 1)
    dma_xc(cs, c8, 0)
    dma_w(wvs, wv8)
    for blk in range(1, 4):
        dma_xc(cs, c8, blk)
    dma_xc(xs, x8, 2)
    dma_xc(xs, x8, 3)
    for t in range(2):
        nc.sync.dma_start(wos[:, t, :], wo[t * 128:(t + 1) * 128, :])
    nc.gpsimd.memset(vs[:, :, :, DH:DH + 1], 1.0)

    # ---- projection chains: 12-instr 3-term hi/lo fp8 DoubleRow ----
    # products: (W_hi,x_hi), (W_lo,x_hi), (W_hi,x_lo); each instr packs the
    # same term for two adjacent k-tiles.
    TERMS = ((0, 0), (1, 0), (0, 1))

    def proj_mms(ps, w, wcols, src, scols):
        n = 0
        for whl, xhl in TERMS:
            for i in range(KT // 2):
                nc.tensor.matmul(
                    ps,
                    lhsT=w[:, 2 * i:2 * i + 2, whl, wcols],
                    rhs=src[:, 2 * i:2 * i + 2, xhl, scols],
                    start=(n == 0), stop=(n == 3 * KT // 2 - 1),
                    perf_mode=DR,
                )
                n += 1

    def q_chain(jb, i4):
        # q for head-pair jb, tokens [i4*512, +512); store exact fp8 hi+lo
        ps = projp.tile([128, 512], F32, tag="proj", name="ps")
        cc = slice(i4 * 512, (i4 + 1) * 512)
        proj_mms(ps[:, :], wqs, slice(jb * 128, (jb + 1) * 128), xs, cc)
        nc.vector.tensor_scalar_mul(qs[:, jb, 0, cc], ps[:, :], SQ)
        nc.vector.scalar_tensor_tensor(qs[:, jb, 1, cc], ps[:, :], SQ,
                                       qs[:, jb, 0, cc], mult, sub)

    def k_chain(jb, i4):
        ps = projp.tile([128, 512], F32, tag="proj", name="ps")
        cc = slice(i4 * 512, (i4 + 1) * 512)
        proj_mms(ps[:, :], wks, slice(jb * 128, (jb + 1) * 128), cs, cc)
        nc.vector.tensor_scalar_mul(ks[:, jb, cc], ps[:, :], SQ)

    def v_chain(tt):
        # v for one context-token tile (token-major): [128 tok, HPC, DH]
        ps = projp.tile([128, HPC, DH], F32, tag="proj", name="ps")
        n = 0
        for whl, xhl in TERMS:
            for i in range(KT // 2):
                nc.tensor.matmul(
                    ps[:, :, :],
                    lhsT=cs[:, 2 * i:2 * i + 2, whl, tt * 128:(tt + 1) * 128],
                    rhs=wvs[:, 2 * i:2 * i + 2, xhl, :],
                    start=(n == 0), stop=(n == 3 * KT // 2 - 1),
                    perf_mode=DR,
                )
                n += 1
        nc.vector.tensor_scalar_mul(vs[:, tt, :, 0:DH], ps[:, :, :], SV)

    # ---- filler queue ----
    queue = deque()

    def final_unit(ib, ob):
        fp = projp.tile([128, 512], F32, tag="proj", name="fp")
        for t2 in range(2):
            nc.tensor.matmul(
                fp[:, :],
                lhsT=wos[:, t2, ob * 128:(ob + 1) * 128],
                rhs=pvs[:, t2, ib * 512:(ib + 1) * 512],
                start=(t2 == 0), stop=(t2 == 1),
            )
        ot = outp.tile([128, 512], F32, tag="ot", name="ot")
        nc.vector.tensor_copy(ot[:, :], fp[:, :])
        nc.sync.dma_start(out[ob * 128:(ob + 1) * 128, ib * 512:(ib + 1) * 512], ot[:, :])

    def push_finals(ib):
        for ob in range(QD // 128):
            queue.append((EST_FIN, lambda ib=ib, ob=ob: final_unit(ib, ob)))

    def make_pv(hp, ib2, after_norms=()):
        # pv mm/norm units for one attn call. The two heads run as strictly
        # sequential phases (all h01=0 tiles, norm, then all h01=1 tiles,
        # norm): the pv PSUM pool only has 2 bufs, and overlapping phases
        # would park >4 PE instructions on the buf-free semaphore, stalling
        # the in-order PE stream (ENG_WAIT_QUEUE_DEPTH=4).
        cell = {}

        def mm(tt, h01, e):
            if tt == 0:
                cell[h01] = [pvp.tile([DH + 1, 512], F32, tag="pv", name="pv")
                             for _ in range(2)]
            for i01 in range(2):
                nc.tensor.matmul(
                    cell[h01][i01][:, :],
                    lhsT=vs[:, tt, 2 * hp + h01, :],
                    rhs=e[:, i01 * 512:(i01 + 1) * 512],
                    start=(tt == 0), stop=(tt == TT - 1),
                )

        def norm(h01):
            for i01 in range(2):
                p = cell[h01][i01]
                c0 = ib2 * 1024 + i01 * 512
                rc = nrm.tile([1, 512], F32, tag="rc", name="rc")
                nc.vector.reciprocal(rc[:, :], p[64:65, :])
                rep = nrm.tile([64, 512], F32, tag="rep", name="rep")
                nc.gpsimd.partition_broadcast(rep[:, :], rc[:, :])
                nc.vector.tensor_tensor(
                    pvs[h01 * 64:(h01 + 1) * 64, hp, c0:c0 + 512],
                    p[0:64, :], rep[:, :], mult)

        def push(tt, es):
            queue.append((EST_PV, lambda tt=tt, e=es[(tt, 0)]: mm(tt, 0, e)))
            if tt == TT - 1:
                # EST_NORM spaces the queue so the h01=1 phase doesn't pop
                # (and park on the pv bufs) before norm(0) has freed them.
                queue.append((EST_NORM, lambda: norm(0)))
                for t2 in range(TT):
                    queue.append((EST_PV, lambda t2=t2, e=es[(t2, 1)]: mm(t2, 1, e)))
                queue.append((EST_NORM, lambda: norm(1)))
                for fn in after_norms:
                    fn()

        return push

    def attn(hp, ib2, push_pv):
        # per ctx tile: 4 one-sided fp8 DoubleRow QK matmuls + 2 exp passes;
        # drain the filler queue between tiles, paced by estimated PE-ns.
        budget = 0.0
        es = {}
        for tt in range(TT):
            qk0 = qkp.tile([128, 1024], F32, tag="qk", name="qk0")
            qk1 = qkp.tile([128, 1024], F32, tag="qk", name="qk1")
            for h01, qk in ((0, qk0), (1, qk1)):
                lhsT = (ks[h01 * 64:(h01 + 1) * 64, hp, tt * 128:(tt + 1) * 128]
                        .unsqueeze(1).broadcast_to([64, 2, 128]))
                for i01 in range(2):
                    c0 = ib2 * 1024 + i01 * 512
                    nc.tensor.matmul(
                        qk[:, i01 * 512:(i01 + 1) * 512],
                        lhsT=lhsT,
                        rhs=qs[h01 * 64:(h01 + 1) * 64, hp, :, c0:c0 + 512],
                        start=True, stop=True,
                        perf_mode=DR,
                    )
            e0 = expp.tile([128, 1024], BF, tag="exp", name="e0")
            nc.scalar.activation(e0[:, :], qk0[:, :], Exp, scale=ESC)
            e1 = expp.tile([128, 1024], BF, tag="exp", name="e1")
            nc.scalar.activation(e1[:, :], qk1[:, :], Exp, scale=ESC)
            es[(tt, 0)], es[(tt, 1)] = e0, e1
            push_pv(tt, es)
            budget += FILL_NS
            while queue and budget > 0:
                est, fn = queue.popleft()
                fn()
                budget -= max(est, 1)

    # ---- schedule ----
    # prologue: only what attn(0,0)'s first tiles need
    q_chain(0, 0)
    q_chain(0, 1)
    k_chain(0, 0)
    # v/k fillers interleaved to match the DMA arrival order of the
    # context column-blocks they read
    for i4 in range(1, 4):
        for tt in range(4 * (i4 - 1), 4 * i4):
            queue.append((EST_VCH, lambda tt=tt: v_chain(tt)))
        queue.append((EST_QKCH, lambda i4=i4: k_chain(0, i4)))
    for tt in range(12, TT):
        queue.append((EST_VCH, lambda tt=tt: v_chain(tt)))
    for i4 in range(2, 4):
        queue.append((EST_QKCH, lambda i4=i4: q_chain(0, i4)))

    attn(0, 0, make_pv(0, 0))
    for i4 in range(2):
        queue.append((EST_QKCH, lambda i4=i4: q_chain(1, i4)))
    for i4 in range(4):
        queue.append((EST_QKCH, lambda i4=i4: k_chain(1, i4)))
    attn(0, 1, make_pv(0, 1))
    for i4 in range(2, 4):
        queue.append((EST_QKCH, lambda i4=i4: q_chain(1, i4)))
    attn(1, 0, make_pv(1, 0, after_norms=(lambda: push_finals(0), lambda: push_finals(1))))
    attn(1, 1, make_pv(1, 1, after_norms=(lambda: push_finals(2), lambda: push_finals(3))))
    while queue:
        est, fn = queue.popleft()
        fn()
    ctx.close()


def _hilo(a):
    """[R, C] f32 -> [R, 2, C] fp8 e4m3 (hi, residual lo)."""
    f8 = ml_dtypes.float8_e4m3
    a = np.ascontiguousarray(a, np.float32)
    h = a.astype(f8)
    l = (a - h.astype(np.float32)).astype(f8)
    return np.ascontiguousarray(np.stack([h, l], axis=1))


def _inputs_for_core(c, x, context, Wq, Wk, Wv, Wo):
    b, g = c // (NCORES // 2), c % (NCORES // 2)
    sl = slice(g * HD, (g + 1) * HD)
    key = ("xc", b)
    if key not in _CACHE:
        _CACHE[key] = (_hilo(x[b].T * SX), _hilo(context[b].T * SX))
    x8b, c8b = _CACHE[key]
    return {
        "x8": x8b,
        "c8": c8b,
        "wq8": _hilo(Wq[:, sl] * SW),
        "wk8": _hilo(Wk[:, sl] * SW),
        "wv8": _hilo(Wv[:, sl] * SW),
        "wo": np.ascontiguousarray(Wo[sl, :]).astype(ml_dtypes.bfloat16),
    }


def kernel(x, context, Wq, Wk, Wv, Wo, bo):
    x = np.asarray(x, np.float32)
    context = np.asarray(context, np.float32)
    if "nc" not in _CACHE:
        _CACHE["nc"] = _build()
    _CACHE.pop(("xc", 0), None)
    _CACHE.pop(("xc", 1), None)
    nc = _CACHE["nc"]
    in_maps = [
        _inputs_for_core(c, x, context, np.asarray(Wq), np.asarray(Wk),
                         np.asarray(Wv), np.asarray(Wo))
        for c in range(NCORES)
    ]
    res = run_bass_kernel_spmd(nc, in_maps, list(range(NCORES))).results
    B = x.shape[0]
    G = NCORES // B
    outp = np.empty((B, NN, QD), np.float32)
    for b in range(B):
        acc = res[b * G]["out"].astype(np.float32)
        for g in range(1, G):
            acc = acc + res[b * G + g]["out"]
        outp[b] = acc.T + np.asarray(bo, np.float32)[None, :]
    return outp
